# revision 1
# baseline (speedup 1.0000x reference)
"""AttnBlock (GroupNorm -> QKV -> 4096x4096 spatial attention -> proj -> residual)
for Trainium2, sharded over 8 NeuronCores.

Sharding: core = (batch b, query-slice s); b = core//4, s = core%4.
Each core computes K/V for its full batch image (redundant across the 4 cores
of a batch) and attention/projection for its 1024-query slice. No collectives.

Host-side input prep (exact, tiny): weight transposes, bias folding
(bo2 = bo + wo@bv), and the GroupNorm per-channel affine A = gamma*rstd,
B = beta - mean*A (per batch) so the device applies GroupNorm as one
fused scale+shift while streaming x.

Device layouts (per core):
  hn, q, k: [c, i] with c on partitions (4 chunks of 128)
  vT:       [j, c] with j on partitions (16 tiles of [128, 512] per half)
  scores^T: [j, i] -> softmax along partition axis j:
            exp via ACT (no max subtraction; |scores| <= ~6 by construction),
            denominator via ones-vector matmul, applied after the output
            projection (division commutes with the channel contraction).
All matmuls run as float32r (tf32-like, full PE rate at N=512).
"""
import numpy as np
import concourse.bacc as bacc
import concourse.bass as bass
import concourse.tile as tile
import concourse.mybir as mybir
from concourse.bass_utils import run_bass_kernel_spmd

F32 = mybir.dt.float32
F32R = mybir.dt.float32r
AF = mybir.ActivationFunctionType
OP = mybir.AluOpType

B, C, H, W = 2, 512, 64, 64
HW = H * W                    # 4096
NCORES = 8
NSLICE = 4                    # query slices per batch
SL = HW // NSLICE             # 1024 query positions per core
NG = 32                       # groups
EPS = 1e-6
CCH = C // 128                # 4 channel chunks
NHALF = 2                     # j halves
JH = HW // NHALF              # 2048 j per half
JB = JH // 512                # 4 j-blocks of 512 per half
JC = JH // 128                # 16 j-chunks of 128 per half
IB = SL // 512                # 2 i-blocks of 512
SCALE = float(C) ** -0.5


def build(reps: int = 1):
    nc = bacc.Bacc("TRN2", target_bir_lowering=False)
    dr = {}
    dr["xf"] = nc.dram_tensor("xf", [C, HW], F32, kind="ExternalInput")
    dr["xs"] = nc.dram_tensor("xs", [C, SL], F32, kind="ExternalInput")
    dr["wqT"] = nc.dram_tensor("wqT", [C, C], F32, kind="ExternalInput")
    dr["wkT"] = nc.dram_tensor("wkT", [C, C], F32, kind="ExternalInput")
    dr["wvT"] = nc.dram_tensor("wvT", [C, C], F32, kind="ExternalInput")
    dr["woT"] = nc.dram_tensor("woT", [C, C], F32, kind="ExternalInput")
    # packed per-channel vectors: ball[p, c*5+k], k in {bq, bk, bo2, A, B}
    dr["ball"] = nc.dram_tensor("ball", [128, CCH * 5], F32, kind="ExternalInput")
    dr["xsTb"] = nc.dram_tensor("xsTb", [SL, C], F32, kind="ExternalInput")
    dr["y"] = nc.dram_tensor("y", [SL, C], F32, kind="ExternalOutput")

    with tile.TileContext(nc) as tc:
        _body(nc, tc, reps, dr)
    nc.finalize()
    return nc


def _body(nc, tc, reps, dr):
    from contextlib import ExitStack
    with ExitStack() as ctx:
        pw = ctx.enter_context(tc.tile_pool(name="pw", bufs=1))
        pc = ctx.enter_context(tc.tile_pool(name="pc", bufs=1))
        pq = ctx.enter_context(tc.tile_pool(name="pq", bufs=1))
        pio = ctx.enter_context(tc.tile_pool(name="pio", bufs=1))
        pdr = ctx.enter_context(tc.tile_pool(name="pdr", bufs=2, space="DRAM"))
        pmm = ctx.enter_context(tc.tile_pool(name="pmm", bufs=3, space="PSUM"))
        patt = ctx.enter_context(tc.tile_pool(name="patt", bufs=1, space="PSUM"))

        ball_t = pc.tile([128, CCH * 5], F32, tag="ball", name="ball")
        nc.sync.dma_start(out=ball_t, in_=dr["ball"][:, :])
        bq_t = [ball_t[:, c * 5 + 0:c * 5 + 1] for c in range(CCH)]
        bk_t = [ball_t[:, c * 5 + 1:c * 5 + 2] for c in range(CCH)]
        bo_t = [ball_t[:, c * 5 + 2:c * 5 + 3] for c in range(CCH)]
        A_t = [ball_t[:, c * 5 + 3:c * 5 + 4] for c in range(CCH)]
        B_t = [ball_t[:, c * 5 + 4:c * 5 + 5] for c in range(CCH)]

        onesf = pc.tile([128, 128], F32, tag="onesf", name="onesf")
        nc.vector.memset(onesf, 1.0)
        ones_r = pc.tile([128, 128], F32R, tag="onesr", name="onesr")
        nc.vector.tensor_copy(ones_r[:, :], onesf[:, :])
        e1f = pc.tile([128, 2], F32, tag="e1f", name="e1f")
        nc.vector.memset(e1f, 0.0)
        nc.vector.memset(e1f[0:1, 0:2], 1.0)
        e1_r = pc.tile([128, 2], F32R, tag="e1r", name="e1r")
        nc.vector.tensor_copy(e1_r[:, :], e1f[:, :])
        # warm the Exp table set while the first DMAs stream in
        warmt = pc.tile([128, 1], F32, tag="warmt", name="warmt")
        nc.scalar.activation(warmt[:, :], onesf[:, 0:1], AF.Exp)

        wk_t = [pw.tile([128, C], F32R, tag=f"wk{c}", name=f"wk{c}") for c in range(CCH)]
        wv_t = [pw.tile([128, C], F32R, tag=f"wv{c}", name=f"wv{c}") for c in range(CCH)]
        wo_t = [pw.tile([128, C], F32R, tag=f"wo{c}", name=f"wo{c}") for c in range(CCH)]

        consts = dict(wk_t=wk_t, wv_t=wv_t, wo_t=wo_t,
                      bq_t=bq_t, bk_t=bk_t, bo_t=bo_t, A_t=A_t, B_t=B_t,
                      ones_r=ones_r, e1_r=e1_r, w_loaded=False)
        for _ in range(reps):
            _attn_once(nc, tc, pw, pc, pq, pio, pmm, patt, pdr, dr, consts)
            consts["w_loaded"] = True


def _attn_once(nc, tc, pw, pc, pq, pio, pmm, patt, pdr, dr, cst):
    xf, xs, y = dr["xf"], dr["xs"], dr["y"]
    wk_t, wv_t, wo_t = cst["wk_t"], cst["wv_t"], cst["wo_t"]
    bq_t, bk_t, bo_t = cst["bq_t"], cst["bk_t"], cst["bo_t"]
    A_t, B_t, ones_r = cst["A_t"], cst["B_t"], cst["ones_r"]
    e1_r = cst["e1_r"]

    # DMA queue order at start: first x block, then wv (vT matmuls run first),
    # then wk
    xb_pre = pio.tile([128, CCH, 512], F32, tag="xb", name="xbpre", bufs=2)
    for ci in range(CCH):
        cs = slice(ci * 128, (ci + 1) * 128)
        nc.sync.dma_start(out=xb_pre[:, ci, :], in_=dr["xf"][cs, 0:512])
        if not cst["w_loaded"]:
            nc.sync.dma_start(out=wv_t[ci], in_=dr["wvT"][cs, :].bitcast(F32R))
    if not cst["w_loaded"]:
        for c in range(CCH):
            cs = slice(c * 128, (c + 1) * 128)
            nc.sync.dma_start(out=wk_t[c], in_=dr["wkT"][cs, :].bitcast(F32R))

    with tc.tile_pool(name="pkv", bufs=1) as pkv, \
         tc.tile_pool(name="pacc", bufs=1) as pacc:
        q_t = [pq.tile([128, SL], F32R, tag=f"q{c}", name=f"q{c}")
               for c in range(CCH)]
        acc_t = [[pacc.tile([128, 512], F32R, tag=f"acc{ib}_{co}",
                            name=f"acc{ib}_{co}") for co in range(CCH)]
                 for ib in range(IB)]
        den_t = [pacc.tile([128, 512], F32R, tag=f"den{ib}", name=f"den{ib}")
                 for ib in range(IB)]
        k_t = [pkv.tile([128, JH], F32R, tag=f"k{c}", name=f"k{c}")
               for c in range(CCH)]
        vt_t = [pkv.tile([128, 512], F32R, tag=f"vt{j}", name=f"vt{j}")
                for j in range(JC)]

        def phase_a2_q():
            with tc.tile_pool(name="phns", bufs=1) as phns:
                wq_t = [phns.tile([128, C], F32R, tag=f"wq{c}", name=f"wq{c}")
                        for c in range(CCH)]
                for c in range(CCH):
                    cs = slice(c * 128, (c + 1) * 128)
                    nc.sync.dma_start(out=wq_t[c],
                                      in_=dr["wqT"][cs, :].bitcast(F32R))
                hns = [phns.tile([128, SL], F32R, tag=f"hns{c}", name=f"hns{c}")
                       for c in range(CCH)]
                for c in range(CCH):
                    cs = slice(c * 128, (c + 1) * 128)
                    xst = pio.tile([128, SL], F32, tag="xs", name="xs", bufs=2)
                    nc.sync.dma_start(out=xst, in_=xs[cs, :])
                    nc.vector.tensor_scalar(
                        out=hns[c][:, :], in0=xst[:, :],
                        scalar1=A_t[c], scalar2=B_t[c], op0=OP.mult, op1=OP.add)
                for ib in range(IB):
                    isl = slice(ib * 512, (ib + 1) * 512)
                    for co in range(CCH):
                        qp = pmm.tile([128, 512], F32, tag="mm", name="mm")
                        for ci in range(CCH):
                            nc.tensor.matmul(
                                qp[:, :], wq_t[ci][:, co * 128:(co + 1) * 128],
                                hns[ci][:, isl], start=(ci == 0),
                                stop=(ci == CCH - 1))
                        nc.vector.tensor_scalar(
                            out=q_t[co][:, isl], in0=qp[:, :],
                            scalar1=bq_t[co], scalar2=None, op0=OP.add)

        def kv_production(h):
            for jb in range(JB):
                if h == 0 and jb == 0:
                    xb = xb_pre
                else:
                    j0 = h * JH + jb * 512
                    xb = pio.tile([128, CCH, 512], F32, tag="xb", name="xb",
                                  bufs=2)
                    nc.sync.dma_start(
                        out=xb,
                        in_=bass.AP(tensor=dr["xf"], offset=j0,
                                    ap=[[HW, 128], [128 * HW, CCH], [1, 512]]))
                hnb = []
                for ci in range(CCH):
                    hb = pio.tile([128, 512], F32R, tag=f"hnb{ci}", name="hnb",
                                  bufs=2)
                    nc.vector.tensor_scalar(
                        out=hb[:, :], in0=xb[:, ci, :],
                        scalar1=A_t[ci], scalar2=B_t[ci], op0=OP.mult, op1=OP.add)
                    hnb.append(hb)
                lsl = slice(jb * 512, (jb + 1) * 512)
                for jt in range(4):
                    vp = pmm.tile([128, 512], F32, tag="mm", name="mm")
                    for ci in range(CCH):
                        nc.tensor.matmul(
                            vp[:, :], hnb[ci][:, jt * 128:(jt + 1) * 128],
                            wv_t[ci][:, :], start=(ci == 0), stop=(ci == CCH - 1))
                    nc.vector.tensor_copy(vt_t[jb * 4 + jt][:, :], vp[:, :])
                for co in range(CCH):
                    kp = pmm.tile([128, 512], F32, tag="mm", name="mm")
                    for ci in range(CCH):
                        nc.tensor.matmul(
                            kp[:, :], wk_t[ci][:, co * 128:(co + 1) * 128],
                            hnb[ci][:, :], start=(ci == 0), stop=(ci == CCH - 1))
                    nc.vector.tensor_scalar(
                        out=k_t[co][:, lsl], in0=kp[:, :],
                        scalar1=bk_t[co], scalar2=None, op0=OP.add)

        def attention(h, ib, mid_emit=None):
            isl = slice(ib * 512, (ib + 1) * 512)
            att_ps = [patt.tile([128, 512], F32, tag=f"att{co}",
                                name=f"att{co}") for co in range(CCH)]
            den_ps = patt.tile([128, 512], F32, tag="den", name="den")

            PIPE = 2  # scores/exp groups emitted ahead of their attnV

            def scores(jc):
                sp = pmm.tile([128, 512], F32, tag="mm", name="mm")
                for ci in range(CCH):
                    nc.tensor.matmul(
                        sp[:, :], k_t[ci][:, jc * 128:(jc + 1) * 128],
                        q_t[ci][:, isl], start=(ci == 0), stop=(ci == CCH - 1))
                eT = pio.tile([128, 512], F32R, tag="eT", name="eT", bufs=4)
                nc.scalar.activation(eT[:, :], sp[:, :], AF.Exp,
                                     bias=0.0, scale=SCALE)
                return eT

            eTs = {jc: scores(jc) for jc in range(PIPE)}
            if mid_emit is not None:
                mid_emit()
            for jc in range(JC):
                if jc + PIPE < JC:
                    eTs[jc + PIPE] = scores(jc + PIPE)
                eT = eTs.pop(jc)
                for co in range(CCH):
                    nc.tensor.matmul(
                        att_ps[co][:, :], vt_t[jc][:, co * 128:(co + 1) * 128],
                        eT[:, :], start=(jc == 0), stop=(jc == JC - 1))
                nc.tensor.matmul(
                    den_ps[:, :], ones_r[:, :], eT[:, :],
                    start=(jc == 0), stop=(jc == JC - 1))
            recT = None
            if h == 0:
                nc.scalar.activation(den_t[ib][:, :], den_ps[:, :], AF.Copy,
                                     bias=0.0, scale=1.0)
            else:
                # den first: the reciprocal chain clears the DVE queue before
                # the accumulator adds, so the fused stores never wait on it
                nc.vector.tensor_add(den_t[ib][:, :],
                                     den_t[ib][:, :].bitcast(F32),
                                     den_ps[:, :])
                recT = rec_chain(ib)
            for co in range(CCH):
                if h == 0:
                    if co < 2:
                        nc.scalar.activation(acc_t[ib][co][:, :],
                                             att_ps[co][:, :], AF.Copy,
                                             bias=0.0, scale=1.0)
                    else:
                        nc.vector.tensor_copy(acc_t[ib][co][:, :],
                                              att_ps[co][:, :])
                else:
                    nc.vector.tensor_add(acc_t[ib][co][:, :],
                                         acc_t[ib][co][:, :].bitcast(F32),
                                         att_ps[co][:, :])
            return recT

        def rec_chain(ib):
            # transpose den onto i-partitions: out[i,0] = den[0, it*128+i] via
            # K=1 matmul with the unit vector, then one tiny approx reciprocal
            dT = patt.tile([128, 4, 2], F32, tag="den", name="dT")
            for it in range(4):
                nc.tensor.matmul(
                    dT[:, it, :],
                    den_t[ib][:, it * 128:(it + 1) * 128],
                    e1_r[:, 0:2], start=True, stop=True,
                    skip_group_check=True)
            recT = pio.tile([128, 4, 2], F32, tag="recT", name="recT", bufs=2)
            nc.vector.reciprocal_approx_fast(out=recT[:, :, :], in_=dT[:, :, :])
            return recT

        def finalize(ib, recT):
            # proj in [i, c] layout: lhsT = acc i-slice, rhs = woT chunk;
            # fin = (pp * recT) + (x_slice^T + bo2)  in one fused DVE op
            for it in range(4):
                rows = slice(ib * 512 + it * 128, ib * 512 + (it + 1) * 128)
                pp = pmm.tile([128, 512], F32, tag="mm", name="mm")
                for idx in range(CCH):
                    ci = (it + idx) % CCH
                    nc.tensor.matmul(
                        pp[:, :],
                        acc_t[ib][ci][:, it * 128:(it + 1) * 128],
                        wo_t[ci][:, :], start=(idx == 0), stop=(idx == CCH - 1))
                xrT = pio.tile([128, 512], F32, tag="xr", name="xr", bufs=3)
                nc.sync.dma_start(out=xrT, in_=dr["xsTb"][rows, :])
                fin = pio.tile([128, 512], F32, tag="fin", name="fin", bufs=2)
                nc.vector.scalar_tensor_tensor(
                    out=fin[:, :], in0=pp[:, :], scalar=recT[:, it, 0:1],
                    in1=xrT[:, :], op0=OP.mult, op1=OP.add)
                nc.sync.dma_start(out=y[rows, :], in_=fin[:, :])

        kv_production(0)
        phase_a2_q()
        if not cst["w_loaded"]:
            for c in range(CCH):
                cs = slice(c * 128, (c + 1) * 128)
                nc.sync.dma_start(out=wo_t[c],
                                  in_=dr["woT"][cs, :].bitcast(F32R))
        attention(0, 0)
        attention(0, 1)
        kv_production(1)
        rb0 = attention(1, 0)
        rb1 = attention(1, 1, mid_emit=lambda: finalize(0, rb0))
        finalize(1, rb1)


_NC_CACHE = {}


def _get_nc(reps: int = 1):
    if reps not in _NC_CACHE:
        _NC_CACHE[reps] = build(reps)
    return _NC_CACHE[reps]


def _host_inputs(x, norm_gamma, norm_beta, wq, bq, wk, bk, wv, bv, wo, bo):
    f32, f64 = np.float32, np.float64
    wqT = np.ascontiguousarray(np.asarray(wq, f32).T)
    wkT = np.ascontiguousarray(np.asarray(wk, f32).T)
    wvT = np.ascontiguousarray(np.asarray(wv, f32).T)
    woT = np.ascontiguousarray(np.asarray(wo, f32).T)
    bo2 = np.asarray(bo, f64) + np.asarray(wo, f64) @ np.asarray(bv, f64)

    x = np.asarray(x, f32)
    gamma = np.asarray(norm_gamma, f64)
    beta = np.asarray(norm_beta, f64)
    shared = {"wqT": wqT, "wkT": wkT, "wvT": wvT, "woT": woT}
    in_maps = []
    for core in range(NCORES):
        b, s = core // NSLICE, core % NSLICE
        xfb = np.ascontiguousarray(x[b].reshape(C, HW))
        xsb = np.ascontiguousarray(xfb[:, s * SL:(s + 1) * SL])
        # GroupNorm affine per channel for this batch (fp64 host stats)
        xg = xfb.astype(f64).reshape(NG, (C // NG) * HW)
        mean = xg.mean(axis=1)
        var = xg.var(axis=1)
        rstd = 1.0 / np.sqrt(var + EPS)
        gmat = gamma.reshape(NG, C // NG)
        Ag = (gmat * rstd[:, None]).reshape(C)
        Bg = (beta.reshape(NG, C // NG)
              - mean[:, None] * gmat * rstd[:, None]).reshape(C)
        ball = np.stack([np.asarray(bq, f64), np.asarray(bk, f64), bo2,
                         Ag, Bg], axis=1)
        ball = ball.reshape(CCH, 128, 5).transpose(1, 0, 2).reshape(128, CCH * 5)
        xsTb = np.ascontiguousarray(xsb.T.astype(f64) + bo2[None, :], f32)
        in_maps.append(dict(shared, xf=xfb, xs=xsb, xsTb=xsTb,
                            ball=np.ascontiguousarray(ball, f32)))
    return in_maps


def kernel(x, norm_gamma, norm_beta, wq, bq, wk, bk, wv, bv, wo, bo,
           reps: int = 1):
    nc = _get_nc(reps)
    in_maps = _host_inputs(x, norm_gamma, norm_beta, wq, bq, wk, bk, wv, bv,
                           wo, bo)
    res = run_bass_kernel_spmd(nc, in_maps, core_ids=list(range(NCORES)),
                               trace=False)
    out = np.empty((B, C, HW), np.float32)
    for core in range(NCORES):
        b, s = core // NSLICE, core % NSLICE
        out[b][:, s * SL:(s + 1) * SL] = res.results[core]["y"].T
    return out.reshape(B, C, H, W)



# revision 41
# speedup vs baseline: 2.9514x; 2.9514x over previous
"""AttnBlock (GroupNorm -> QKV -> 4096x4096 spatial attention -> proj -> residual)
for Trainium2, sharded over 8 NeuronCores.

Sharding: core = (batch b, query-slice s); b = core//4, s = core%4. Each core
computes V for its full batch image (redundant across the 4 cores of a batch)
and attention/projection for its 1024-query slice. No collectives. The host
rolls the j-columns of each core's x so the core's own query slice is always
j-blocks 0..1 -> the SPMD program is identical across cores (softmax over j is
permutation-invariant).

All heavy matmuls run in fp8 (e4m3) with MatmulPerfMode.DoubleRow: operands
laid out [128p, 2, free] (contraction K=256 per instruction) at 0.5
cycles/row -- 4x the f32r rate of the fp32 version. The entire GroupNorm +
QK^T path is algebraically folded on the host (exact up to fp8 rounding):

  hn = A*x + B               (A,B from exact fp64 host stats, as baseline)
  scores^T = hn^T wk^T (wq hn + bq)
           = x8^T * [A-row-scaled WQK] * x8 + (i-const -> dropped: softmax
             over j is invariant to i-dependent shifts), WQK = (wk^T wq)^T A
  v = wv hn + bv -> [A-scaled WV] x8 + const -> const folds into residual
             (sum_j attn = 1), bo2 = bo + wo @ (bv + wv @ B)

so the device computes: q~ = WQK8^T x8 (+A*tq bias), V = WV8^T x8,
scores = x8^T q~8, exp (scale=1/sqrt(C), output 0.25*e^s in fp8, max ~100 <
240, no max-subtraction needed since |scores| <= ~6), attV + denominator
(DoubleRow ones-matmul) accumulated in PSUM over all 4096 j, out-proj in fp8,
then one fused divide+residual DVE op. x ships as fp8 straight from host; no
hn/k tensors are ever materialized. Weight prescale 32 and the attn-acc
prescale 1/16 fold into the final reciprocal (rec = 1/(2*den)).
"""
import numpy as np
import ml_dtypes
import concourse.bacc as bacc
import concourse.bass as bass
import concourse.tile as tile
import concourse.mybir as mybir
from concourse.bass_utils import run_bass_kernel_spmd

F32 = mybir.dt.float32
F32R = mybir.dt.float32r
FP8 = mybir.dt.float8e4
AF = mybir.ActivationFunctionType
OP = mybir.AluOpType
DR = mybir.MatmulPerfMode.DoubleRow

B, C, H, W = 2, 512, 64, 64
HW = H * W                    # 4096
NCORES = 8
NSLICE = 4                    # query slices per batch
SL = HW // NSLICE             # 1024 query positions per core
NG = 32                      # groups
EPS = 1e-6
CCH = C // 128                # 4 channel chunks
NH = 2                        # channel halves (256 each)
JBN = HW // 512               # 8 j-blocks
JPN = HW // 256               # 16 j-pairs
IB = SL // 512                # 2 i-blocks
SCALE = float(C) ** -0.5
WS = 32.0                     # host weight prescale before fp8 quantization
ALPHA = 0.25                  # exp output scale (keeps eT <= ~100 < 240)
ACCS = 1.0 / 16.0             # attn-accumulator -> fp8 scale
E1V = WS * ACCS               # folded into the reciprocal (rec = 1/(E1V*den))
LN_ALPHA = float(np.log(ALPHA))


def build(reps: int = 1):
    nc = bacc.Bacc("TRN2", target_bir_lowering=False)
    dr = {}
    dr["xpk"] = nc.dram_tensor("xpk", [128, JBN * CCH * 512], FP8,
                               kind="ExternalInput")
    for w in ("wqk8", "wv8", "wo8"):
        dr[w] = nc.dram_tensor(w, [128, NH * 2 * 512], FP8, kind="ExternalInput")
    # packed per-channel vectors: ball[p, ci*2+k], k in {A/WS, A*tq}
    dr["ball"] = nc.dram_tensor("ball", [128, CCH * 2], F32, kind="ExternalInput")
    dr["xr"] = nc.dram_tensor("xr", [SL, C], F32, kind="ExternalInput")
    dr["y"] = nc.dram_tensor("y", [SL, C], F32, kind="ExternalOutput")

    with tile.TileContext(nc) as tc:
        _body(nc, tc, reps, dr)
    nc.finalize()
    return nc


def _body(nc, tc, reps, dr):
    from contextlib import ExitStack
    with ExitStack() as ctx:
        pc = ctx.enter_context(tc.tile_pool(name="pc", bufs=1))
        pw = ctx.enter_context(tc.tile_pool(name="pw", bufs=1))
        px = ctx.enter_context(tc.tile_pool(name="px", bufs=1))
        pv = ctx.enter_context(tc.tile_pool(name="pv", bufs=1))
        pq = ctx.enter_context(tc.tile_pool(name="pq", bufs=1))
        pacc = ctx.enter_context(tc.tile_pool(name="pacc", bufs=1))
        pio = ctx.enter_context(tc.tile_pool(name="pio", bufs=1))
        pmm = ctx.enter_context(tc.tile_pool(name="pmm", bufs=4, space="PSUM"))
        patt = ctx.enter_context(tc.tile_pool(name="patt", bufs=1, space="PSUM"))

        ball_t = pc.tile([128, CCH * 2], F32, tag="ball", name="ball")
        nc.gpsimd.dma_start(out=ball_t, in_=dr["ball"][:, :])
        aq1 = [ball_t[:, c * 2 + 0:c * 2 + 1] for c in range(CCH)]
        aq2 = [ball_t[:, c * 2 + 1:c * 2 + 2] for c in range(CCH)]

        ones8 = pc.tile([128, 2, 2], FP8, tag="ones8", name="ones8")
        nc.vector.memset(ones8, E1V)
        lnat = pc.tile([128, 1], F32, tag="lnat", name="lnat")
        nc.vector.memset(lnat, LN_ALPHA)
        # warm the Exp table set while the first DMAs stream in
        warmt = pc.tile([128, 1], F32, tag="warmt", name="warmt")
        nc.scalar.activation(warmt[:, :], lnat[:, :], AF.Exp, bias=lnat[:, 0:1])

        x8 = [px.tile([128, CCH, 512], FP8, tag=f"x{jb}", name=f"x{jb}")
              for jb in range(JBN)]
        xr_t = pio.tile([128, IB * 4, 512], F32, tag="xr", name="xr")

        def ldx(jb):
            nc.sync.dma_start(
                out=x8[jb],
                in_=bass.AP(tensor=dr["xpk"], offset=jb * 2048,
                            ap=[[JBN * 2048, 128], [512, CCH], [1, 512]]))

        ldx(0)
        wt = {}
        for w in ("wv8", "wqk8", "wo8"):
            t = pw.tile([128, NH, 2, 512], FP8, tag=w, name=w)
            if w != "wo8":
                nc.sync.dma_start(
                    out=t,
                    in_=bass.AP(tensor=dr[w], offset=0,
                                ap=[[2048, 128], [1024, NH], [512, 2], [1, 512]]))
            wt[w] = t
        ldx(1)

        def ld_late():
            # wo / xr are first needed ~mid-kernel; keep them off the DMA
            # engines while the x blocks stream in
            nc.sync.dma_start(
                out=wt["wo8"],
                in_=bass.AP(tensor=dr["wo8"], offset=0,
                            ap=[[2048, 128], [1024, NH], [512, 2], [1, 512]]))
            nc.sync.dma_start(
                out=xr_t,
                in_=bass.AP(tensor=dr["xr"], offset=0,
                            ap=[[C, 128], [128 * C, IB * 4], [1, C]]))

        consts = dict(wt=wt, aq1=aq1, aq2=aq2, ones8=ones8,
                      lnat=lnat, x8=x8, xr_t=xr_t, ldx=ldx, ld_late=ld_late,
                      first=True)
        for _ in range(reps):
            _attn_once(nc, tc, pv, pq, pacc, pio, pmm, patt, dr, consts)
            consts["first"] = False


def _attn_once(nc, tc, pv, pq, pacc, pio, pmm, patt, dr, cst):
    y = dr["y"]
    wqk_t, wv_t, wo_t = (cst["wt"][k] for k in ("wqk8", "wv8", "wo8"))
    aq1, aq2 = cst["aq1"], cst["aq2"]
    ones8, lnat = cst["ones8"], cst["lnat"]
    x8, xr_t, ldx = cst["x8"], cst["xr_t"], cst["ldx"]

    vt = [pv.tile([128, 2, 512], FP8, tag=f"vt{jp}", name=f"vt{jp}")
          for jp in range(JPN)]
    qt8 = [[pq.tile([128, 2, 512], FP8, tag=f"qt{h}_{ib}", name=f"qt{h}_{ib}")
            for ib in range(IB)] for h in range(NH)]
    acc8 = [[pacc.tile([128, 2, 512], FP8, tag=f"acc{ib}_{h}",
                       name=f"acc{ib}_{h}") for h in range(NH)]
            for ib in range(IB)]
    recT = [pacc.tile([128, 8], F32, tag=f"rec{ib}", name=f"rec{ib}")
            for ib in range(IB)]
    denT = [pacc.tile([128, 8], F32, tag=f"dent{ib}", name=f"dent{ib}")
            for ib in range(IB)]

    def produce(jb, jts=(0, 1, 2, 3), load=False):
        """Project V j-chunks of block jb (PE + one DVE fp8 copy per 128 j)."""
        if load and (jb >= 2 or not cst["first"]):
            ldx(jb)
        for jt in jts:
            vp = pmm.tile([128, 512], F32, tag="mm", name="mm")
            for h in range(NH):
                nc.tensor.matmul(
                    vp[:, :], x8[jb][:, 2 * h:2 * h + 2, jt * 128:(jt + 1) * 128],
                    wv_t[:, h, :, :], start=(h == 0), stop=(h == NH - 1),
                    perf_mode=DR)
            if (jb < 6 and jt == 1) or (jb < 2 and jt == 3):
                # DVE is the produce-phase bottleneck; ACT (exp-paced with
                # slack until the ib0 tail) absorbs some v conversions
                nc.scalar.activation(
                    out=vt[jb * 2 + jt // 2][:, jt % 2, :], in_=vp[:, :],
                    func=AF.Copy, bias=0.0, scale=1.0 / WS)
            else:
                nc.vector.tensor_scalar(
                    out=vt[jb * 2 + jt // 2][:, jt % 2, :], in0=vp[:, :],
                    scalar1=1.0 / WS, scalar2=None, op0=OP.mult)

    def q_tilde(ib):
        """q~ = A * (WQK8^T x8 / WS + tq), the only query-side projection."""
        for ci in range(CCH):
            tp = pmm.tile([128, 512], F32, tag="mm", name="mm")
            for h in range(NH):
                nc.tensor.matmul(
                    tp[:, :], wqk_t[:, h, :, ci * 128:(ci + 1) * 128],
                    x8[ib][:, 2 * h:2 * h + 2, :],
                    start=(h == 0), stop=(h == NH - 1), perf_mode=DR)
            nc.vector.tensor_scalar(
                out=qt8[ci // 2][ib][:, ci % 2, :], in0=tp[:, :],
                scalar1=aq1[ci], scalar2=aq2[ci], op0=OP.mult, op1=OP.add)

    def scores_pair(ib, jb, jl):
        """One j-pair of scores+exp; returns (ib, jp, eT, spt) for later attV.
        spt is the t=1 scores PSUM tile: once its exp consumed it, the pair's
        tiny transposed-den matmuls reuse its bank (no dedicated den bank)."""
        jp = jb * 2 + jl // 2
        eT = pio.tile([128, 2, 512], FP8, tag="eT", name="eT", bufs=8)
        spt = None
        for t in range(2):
            sp = pmm.tile([128, 512], F32, tag="mm", name="mm")
            for h in range(NH):
                nc.tensor.matmul(
                    sp[:, :],
                    x8[jb][:, 2 * h:2 * h + 2,
                           (jl + t) * 128:(jl + t + 1) * 128],
                    qt8[h][ib][:, :, :],
                    start=(h == 0), stop=(h == NH - 1), perf_mode=DR)
            nc.scalar.activation(eT[:, t, :], sp[:, :], AF.Exp,
                                 bias=lnat[:, 0:1], scale=SCALE)
            spt = sp
        return ib, jp, eT, spt

    def attv(ib, jp, eT, spt, att):
        for co in range(CCH):
            nc.tensor.matmul(
                att[:, co, :], vt[jp][:, :, co * 128:(co + 1) * 128],
                eT[:, :, :], start=(jp == 0), stop=(jp == JPN - 1),
                perf_mode=DR)
        # denT[i, :] += E1V * sum_j eT[j, :, i] via N=2 matmuls into the dead
        # scores bank, accumulated on DVE (frees a PSUM bank for the mm pool)
        for it in range(4):
            nc.tensor.matmul(
                spt[:, it * 2:(it + 1) * 2],
                eT[:, :, it * 128:(it + 1) * 128], ones8[:, :, :],
                start=True, stop=True, skip_group_check=True, perf_mode=DR)
        if jp == 0:
            nc.vector.tensor_copy(denT[ib][:, 0:8], spt[:, 0:8])
        else:
            nc.vector.tensor_add(denT[ib][:, 0:8], denT[ib][:, 0:8],
                                 spt[:, 0:8])
        if jp == JPN - 1:
            nc.vector.reciprocal_approx_fast(out=recT[ib][:, 0:8],
                                             in_=denT[ib][:, 0:8])

    pend = []

    def emit(ib, jb, jl=None):
        for l in ((0, 2) if jl is None else (jl,)):
            pend.append(scores_pair(ib, jb, l))

    def drain(n, att):
        for _ in range(n):
            attv(*pend.pop(0), att=att)

    def fin_copies(ib, att, acc_act=0):
        """PSUM attn accumulators -> fp8. DVE/ACT only -- no PE ops, so the
        in-order PE queue keeps streaming the next i-block."""
        for h in range(NH):
            if h < acc_act:
                nc.scalar.activation(
                    out=acc8[ib][h][:, :, :], in_=att[:, 2 * h:2 * h + 2, :],
                    func=AF.Copy, bias=0.0, scale=ACCS)
            else:
                nc.vector.tensor_scalar(
                    out=acc8[ib][h][:, :, :], in0=att[:, 2 * h:2 * h + 2, :],
                    scalar1=ACCS, scalar2=None, op0=OP.mult)

    def proj(ib, its=(0, 1, 2, 3)):
        """out-proj in [i, c] layout, fused divide + residual, store."""
        for it in its:
            rows = slice(ib * 512 + it * 128, ib * 512 + (it + 1) * 128)
            pp = pmm.tile([128, 512], F32, tag="mm", name="mm")
            for h in range(NH):
                nc.tensor.matmul(
                    pp[:, :], acc8[ib][h][:, :, it * 128:(it + 1) * 128],
                    wo_t[:, h, :, :], start=(h == 0), stop=(h == NH - 1),
                    perf_mode=DR)
            fin = pio.tile([128, 512], F32, tag="fin", name="fin", bufs=4)
            nc.vector.scalar_tensor_tensor(
                out=fin[:, :], in0=pp[:, :],
                scalar=recT[ib][:, it * 2:it * 2 + 1],
                in1=xr_t[:, ib * 4 + it, :], op0=OP.mult, op1=OP.add)
            nc.sync.dma_start(out=y[rows, :], in_=fin[:, :])

    # software-pipelined attention: scores/exp run 2+ j-pairs ahead of the
    # attV accumulation, and V-production interleaves with score pairs, so
    # the in-order PE and ACT queues never ping-pong
    att0 = patt.tile([128, CCH, 512], F32, tag="att", name="att0")

    produce(0, load=True)
    q_tilde(0)
    produce(1, load=True)
    emit(0, 0)
    q_tilde(1)
    for jb in range(2, JBN):
        produce(jb, jts=(), load=True)     # just the ldx
        emit(0, jb - 1, 0)
        produce(jb, jts=(0, 1))
        emit(0, jb - 1, 2)
        produce(jb, jts=(2, 3))
        drain(2, att0)
    if cst["first"]:
        cst["ld_late"]()
    emit(0, JBN - 1)
    drain(2, att0)

    # i-block boundary: PSUM->fp8 copies are DVE/ACT-only; keep j-blocks of
    # ib1 scores/exp as a PE/ACT runway before the first attV needs the
    # (reused) accumulator banks
    att1 = patt.tile([128, CCH, 512], F32, tag="att", name="att1")
    emit(1, 0)
    drain(2, att0)        # ib0 closes
    fin_copies(0, att0, acc_act=0)
    emit(1, 1)
    emit(1, 2)
    drain(2, att1)        # ib1 opens (after the acc8 copies)
    for jb in range(3, JBN):
        emit(1, jb)
        drain(2, att1)
        if jb >= 4:
            proj(0, its=(jb - 4,))
    drain(4, att1)
    fin_copies(1, att1, acc_act=2)
    proj(1)


_NC_CACHE = {}


def _get_nc(reps: int = 1):
    if reps not in _NC_CACHE:
        _NC_CACHE[reps] = build(reps)
    return _NC_CACHE[reps]


def _host_inputs(x, norm_gamma, norm_beta, wq, bq, wk, bk, wv, bv, wo, bo):
    f32, f64 = np.float32, np.float64
    E4 = ml_dtypes.float8_e4m3

    def pack_w(wT):  # wT: [Cin-like, Cout-like] -> [128, 2048] with
        # [p, h*1024 + t*512 + n] = WS * wT[h*256 + t*128 + p, n] in fp8
        a = np.asarray(WS * wT, f32).reshape(NH, 2, 128, 512)
        return np.ascontiguousarray(
            a.transpose(2, 0, 1, 3).reshape(128, 2048)).astype(E4)

    wq, wk, wv, wo = [np.asarray(w, f64) for w in (wq, wk, wv, wo)]
    bq, bv, bo = [np.asarray(v, f64) for v in (bq, bv, bo)]
    G = wk.T @ wq                                   # [ci, cj]
    wo8 = pack_w(wo.T)                              # rhs[co, c] = wo[c, co]
    x = np.asarray(x, f32)
    gamma = np.asarray(norm_gamma, f64)
    beta = np.asarray(norm_beta, f64)
    in_maps = []
    per_batch = {}
    for core in range(NCORES):
        b, s = core // NSLICE, core % NSLICE
        if b not in per_batch:
            xfb = np.ascontiguousarray(x[b].reshape(C, HW))
            # GroupNorm affine per channel for this batch (fp64 host stats)
            xg = xfb.astype(f64).reshape(NG, (C // NG) * HW)
            mean = xg.mean(axis=1)
            var = xg.var(axis=1)
            rstd = 1.0 / np.sqrt(var + EPS)
            gmat = gamma.reshape(NG, C // NG)
            Ag = (gmat * rstd[:, None]).reshape(C)
            Bg = (beta.reshape(NG, C // NG)
                  - mean[:, None] * gmat * rstd[:, None]).reshape(C)
            wqk8 = pack_w((G * Ag[None, :]).T)      # lhsT[cj, ci]
            wv8 = pack_w((wv * Ag[None, :]).T)      # rhs[cj, c]
            tq = G @ Bg + wk.T @ bq
            bo2 = bo + wo @ (bv + wv @ Bg)
            ball = np.stack([Ag / WS, Ag * tq], axis=1)
            ball = ball.reshape(CCH, 128, 2).transpose(1, 0, 2).reshape(128, 8)
            per_batch[b] = (xfb, wqk8, wv8, bo2,
                            np.ascontiguousarray(ball, f32))
        xfb, wqk8, wv8, bo2, ball = per_batch[b]
        # roll j so this core's query slice is j-blocks 0..1 (softmax over j
        # is permutation-invariant); pack to [128, jb, ci, 512] fp8
        xro = np.concatenate([xfb[:, s * SL:], xfb[:, :s * SL]], axis=1)
        xpk = np.ascontiguousarray(
            xro.reshape(CCH, 128, JBN, 512).transpose(1, 2, 0, 3)
            .reshape(128, JBN * CCH * 512)).astype(E4)
        xrr = np.ascontiguousarray(
            (xfb[:, s * SL:(s + 1) * SL].T.astype(f64) + bo2[None, :]), f32)
        in_maps.append(dict(xpk=xpk, xr=xrr, ball=ball,
                            wqk8=wqk8, wv8=wv8, wo8=wo8))
    return in_maps


def kernel(x, norm_gamma, norm_beta, wq, bq, wk, bk, wv, bv, wo, bo,
           reps: int = 1):
    nc = _get_nc(reps)
    in_maps = _host_inputs(x, norm_gamma, norm_beta, wq, bq, wk, bk, wv, bv,
                           wo, bo)
    res = run_bass_kernel_spmd(nc, in_maps, core_ids=list(range(NCORES)),
                               trace=False)
    out = np.empty((B, C, HW), np.float32)
    for core in range(NCORES):
        b, s = core // NSLICE, core % NSLICE
        out[b][:, s * SL:(s + 1) * SL] = res.results[core]["y"].T
    return out.reshape(B, C, H, W)
